# revision 24
# baseline (speedup 1.0000x reference)
"""MMD (Maximum Mean Discrepancy) loss kernel for Trainium2, 8 NeuronCores.

Math: with x = concat(source, target) [N=8192, D=256],
  L2_ij = sq_i + sq_j - 2 x_i.x_j
  bandwidth = sum(L2) / (N^2-N) / 4   (closed form: sum(L2) = 2N*sum(sq) - 2||colsum x||^2)
  K = sum_b exp(-L2 / (bandwidth * 2^b)), b = 0..4
  loss = mean(K_SS) + mean(K_TT) - 2 mean(K_ST)

Strategy (triangle sharding over 512x512 tiles; K is symmetric so only the
upper triangle of the 16x16 tile grid is computed — 136 tiles instead of 256):
  total = sum_SS + sum_TT - 2 sum_ST.  Core k owns 17 tiles: SS row-block k
  (diag w=+1, 7-k uppers w=+2), TT row-block 7-k (diag w=+1, k uppers w=+2),
  ST row-block k (8 tiles, w=-2).  Same instruction count per core (SPMD);
  all per-core structure lives in host-packed tensors.

The 5 bandwidths are a geometric ladder (a_{b+1} = a_b/2), so with
e4 = exp(-a_4 L2) every other kernel is a square: e_b = e_{b+1}^2.  Only ONE
exp pass is needed; the rest are element squarings spread over three engines:
  PE:  G_ij = x_i.x_j - 0.5 sq_i - 0.5 sq_j  (= -L2/2): per 128x512 slab two
       fp16 K=128 matmuls (x data; G error ~1e-2 abs -> ~1e-5 on e4) plus one
       f32r K=2 matmul contracting two augmented rows (exact -0.5 sq terms).
       Reduce-matmuls: lhsT = w_t*ones [128,1] contract fp16 value tiles into
       a persistent PSUM accumulator bank; chunk c runs COLUMN-TILED
       (tile_position=(0, 32*(c%4)), 128x32 mode) so 4 reduce MMs execute
       concurrently in the array — out rows {0,32,64,96}, start/stop
       tracked per column group (PSUM accumulate-bits are per-quadrant).
  ACT: e4 = exp(2 a4 G) from PSUM (FD=1024 per pt half, free accum -> b4),
       e2 = Square(e3) per PAIR (FD=4096 SBUF, free accum -> b2).
  DVE: e3 = e4*e4, e1 = e2*e2, e0 = e1*e1 as plain fp16 tensor_tensor in
       2x_1p mode (2 elem/cycle/lane), one op per PAIR of tiles (FD=4096)
       to amortize the per-op startup + drain; the fused reduce variants
       only have 1x uops, so sums ride the PE/ACT instead.
Tiles are processed in same-weight PAIRS (diag pair, 3 upper pairs, 1 upper
single, 4 ST pairs) so pair-granular accums can share weight slots.
Value tiles are fp16 (not bf16): the chained squarings feed later sums, and
the loss is a small difference of large block means; fp16 keeps rounding
noise ~4x lower.  Engine load/tile ~ ACT 4.1us, DVE 3.9us, PE 3.4us (vs
8.0us ACT-bound baseline).  Emission is software-pipelined (reduces lag 1-2
pairs) so no engine head-of-line blocks.
Host computes the bandwidth in closed form (fp64), packs per-core tiles,
applies tile weights, divides by B^2.
"""

import numpy as np

B = 4096
D = 256
N = 2 * B
KERNEL_MUL = 2.0
KERNEL_NUM = 5
NCORES = 8
TS = 512  # tile edge
NTILES = 17  # tiles per core
NIB = 4  # 128-row sub-blocks per tile
NWB = 7  # class-B tiles (t=2..8) with dedicated weights
NUSLOT = 8 + NWB * NIB  # distinct (slab, ib) u-row slots: A(8) + B(28)
NRES = 4  # res slots per tile: b4 (x2 halves), b2 (pair), b1 (pair, amr mode)
E1_VIA_PE = True  # False: e1 sum via affine_mul_reduce (DVE 1x) instead
GROUPS = [(0, 1), (2, 3), (4, 5), (6, 7), (8,), (9, 10), (11, 12), (13, 14), (15, 16)]
GROUPS_SINGLE = [(t,) for t in range(NTILES)]

PROBE = {"no_red": False, "no_vsq": False, "no_sq2": False}
SQ_ON_V = frozenset()  # groups whose e2 square runs on DVE (b2 via PE red)
HINT_PE = True  # branch-hint the For_i back-edge for the PE (>256 instrs/iter)

_CACHE = {}


def _uslot(t, ib):
    """Unit -> slot in the deduplicated u-region of aug2."""
    if t < 2:
        return t * NIB + ib  # A: SSd -> P slots 0-3, TTd -> Q slots 4-7
    if t <= 8:
        return 8 + (t - 2) * NIB + ib  # B: per-tile slots
    return ib  # C (ST): slab P == slots 0-3


def _wclass(t):
    """Tile weight class: 0 -> +1 (diag), 1 -> +2 (upper), 2 -> -2 (ST)."""
    return 0 if t < 2 else (1 if t <= 8 else 2)


def _build_program(repeat=1):
    """Build the SPMD program. repeat>1 wraps the compute body in a hardware
    For loop (identical result; used only for differential HW timing)."""
    import concourse.bass as bass
    import concourse.tile as tile
    from concourse import bacc, mybir

    f32 = mybir.dt.float32
    f32r = mybir.dt.float32r
    f16 = mybir.dt.float16
    Exp = mybir.ActivationFunctionType.Exp
    Square = mybir.ActivationFunctionType.Square

    nc = bacc.Bacc(None)

    xT = nc.declare_dram_parameter("xT", [128, NTILES, 2, TS], f16, isOutput=False)
    wT = nc.declare_dram_parameter("wT", [128, NWB * NIB, 2, 128], f16, isOutput=False)
    # aug2 row layout: cols [0, NUSLOT*128): (ones, u_i) per u-slot;
    # cols [NUSLOT*128, +NTILES*TS): (v_j, ones) per tile.
    AUGW = NUSLOT * 128 + NTILES * TS
    aug = nc.declare_dram_parameter("aug2", [2, AUGW], f32r, isOutput=False)
    scl = nc.declare_dram_parameter("scale", [128, 1], f32, isOutput=False)
    # reduce weights: 3 classes of [128, 1] fp16 w*ones columns
    rw = nc.declare_dram_parameter("rw", [128, 3], f16, isOutput=False)
    res = nc.declare_dram_parameter("res", [128, NTILES * NRES], f32, isOutput=True)
    accd = nc.declare_dram_parameter("accd", [128, TS], f32, isOutput=True)

    with tile.TileContext(nc) as tc:
        with (
            tc.tile_pool(name="sing", bufs=1) as sing,
            tc.tile_pool(name="scr2", bufs=2) as scr2,
            tc.tile_pool(name="scr3", bufs=3) as scr3,
            tc.tile_pool(name="psum", bufs=3, space=bass.MemorySpace.PSUM) as psum,
            tc.tile_pool(name="pacc", bufs=1, space=bass.MemorySpace.PSUM) as pacc,
        ):
            rhs_sb = sing.tile([128, NTILES, 2, TS], f16)
            w_sb = sing.tile([128, NWB * NIB, 2, 128], f16)
            aug_sb = sing.tile([2, AUGW], f32r)
            scale_sb = sing.tile([128, 1], f32)
            rw_sb = sing.tile([128, 3], f16)
            res_sb = sing.tile([128, NTILES * NRES], f32)
            acc_sb = sing.tile([128, TS], f32)
            acc_ps = pacc.tile([128, TS], f32)

            nc.sync.dma_start(out=scale_sb, in_=scl[:])
            nc.sync.dma_start(out=rw_sb, in_=rw[:])
            nc.sync.dma_start(out=aug_sb, in_=aug[:])
            for t in range(NTILES):
                nc.sync.dma_start(out=rhs_sb[:, t], in_=xT[:, t])
                if 2 <= t <= 8:
                    nc.sync.dma_start(
                        out=w_sb[:, (t - 2) * NIB : (t - 1) * NIB],
                        in_=wT[:, (t - 2) * NIB : (t - 1) * NIB],
                    )

            def body():
                pth = {}
                e4v, e3v, e2v, e1v, e0v = {}, {}, {}, {}, {}
                state = {"first": [True] * NIB}

                def emit_mains(g):
                    for ti, t in enumerate(GROUPS[g]):
                        for h in (0, 1):
                            pt = psum.tile([128, 2 * TS], f32, tag="pt")
                            pth[(t, h)] = pt
                            for i2 in (0, 1):
                                ib = 2 * h + i2
                                sl = pt[:, i2 * TS : (i2 + 1) * TS]
                                if t < 2:
                                    lhs0 = rhs_sb[:, t, 0, ib * 128 : (ib + 1) * 128]
                                    lhs1 = rhs_sb[:, t, 1, ib * 128 : (ib + 1) * 128]
                                elif t <= 8:
                                    lhs0 = w_sb[:, (t - 2) * NIB + ib, 0]
                                    lhs1 = w_sb[:, (t - 2) * NIB + ib, 1]
                                else:  # ST: slab-P rows == tile-0 columns
                                    lhs0 = rhs_sb[:, 0, 0, ib * 128 : (ib + 1) * 128]
                                    lhs1 = rhs_sb[:, 0, 1, ib * 128 : (ib + 1) * 128]
                                us = _uslot(t, ib)
                                nc.tensor.matmul(sl, lhs0, rhs_sb[:, t, 0], start=True, stop=False)
                                nc.tensor.matmul(sl, lhs1, rhs_sb[:, t, 1], start=False, stop=False)
                                nc.tensor.matmul(
                                    sl,
                                    aug_sb[:, us * 128 : (us + 1) * 128],
                                    aug_sb[:, NUSLOT * 128 + t * TS : NUSLOT * 128 + (t + 1) * TS],
                                    start=False,
                                    stop=True,
                                )

                def emit_exp(g):
                    m = len(GROUPS[g])
                    ev = scr3.tile([128, m * NIB * TS], f16, tag="e4")
                    e4v[g] = ev
                    for ti, t in enumerate(GROUPS[g]):
                        for h in (0, 1):
                            q = 2 * ti + h
                            nc.scalar.activation(
                                out=ev[:, q * 2 * TS : (q + 1) * 2 * TS],
                                in_=pth.pop((t, h))[:],
                                func=Exp,
                                scale=scale_sb[:, 0:1],
                                accum_out=res_sb[:, t * NRES + h : t * NRES + h + 1],
                            )

                def emit_e3(g):
                    m = len(GROUPS[g])
                    e3 = scr2.tile([128, m * NIB * TS], f16, tag="e3")
                    e3v[g] = e3
                    if not PROBE["no_vsq"]:
                        nc.vector.tensor_mul(e3[:], e4v[g][:], e4v[g][:])

                def emit_sq2(g):
                    m = len(GROUPS[g])
                    t0 = GROUPS[g][0]
                    e2 = scr2.tile([128, m * NIB * TS], f16, tag="e2")
                    e2v[g] = e2
                    if PROBE["no_sq2"]:
                        return
                    if g in SQ_ON_V:
                        nc.vector.tensor_mul(e2[:], e3v[g][:], e3v[g][:])
                    else:
                        nc.scalar.activation(
                            out=e2[:],
                            in_=e3v[g][:],
                            func=Square,
                            accum_out=res_sb[:, t0 * NRES + 2 : t0 * NRES + 3],
                        )

                def emit_e1(g):
                    m = len(GROUPS[g])
                    t0 = GROUPS[g][0]
                    e1 = scr2.tile([128, m * NIB * TS], f16, tag="e1")
                    e1v[g] = e1
                    if PROBE["no_vsq"]:
                        return
                    if E1_VIA_PE:
                        nc.vector.tensor_mul(e1[:], e2v[g][:], e2v[g][:])
                    else:
                        nc.vector.affine_mul_reduce(
                            out=e1[:],
                            accum_out=res_sb[:, t0 * NRES + 3 : t0 * NRES + 4],
                            in0=e2v[g][:],
                            in1=e2v[g][:],
                            scale=1.0,
                            bias=0.0,
                        )

                def emit_e0(g):
                    m = len(GROUPS[g])
                    e0 = scr3.tile([128, m * NIB * TS], f16, tag="e4")
                    e0v[g] = e0
                    if not PROBE["no_vsq"]:
                        nc.vector.tensor_mul(e0[:], e1v[g][:], e1v[g][:])

                def emit_red(g, val, slot):
                    # Column-tiled (128x32 mode): chunk c runs in PE column
                    # group c%4, out -> PSUM partition 32*(c%4); 4 chunks
                    # execute concurrently.  lhsT = w * ones [128, 1].
                    if PROBE["no_red"]:
                        return
                    c = _wclass(GROUPS[g][0])
                    lhs = rw_sb[:, c : c + 1]
                    last = slot == 2 and g == len(GROUPS) - 1
                    nch = len(GROUPS[g]) * NIB
                    for ch in range(nch):
                        k = ch % NIB
                        nc.tensor.matmul(
                            acc_ps[32 * k : 32 * k + 1, :],
                            lhs,
                            val[:, ch * TS : (ch + 1) * TS],
                            start=state["first"][k],
                            stop=last and ch >= nch - NIB,
                            skip_group_check=True,
                            tile_position=(0, 32 * k),
                        )
                        state["first"][k] = False

                NG = len(GROUPS)
                for u in range(NG + 2):
                    if u < NG:
                        emit_mains(u)
                        emit_exp(u)
                        emit_e3(u)
                    if 1 <= u <= NG:
                        g = u - 1
                        emit_sq2(g)
                        emit_e1(g)
                        emit_e0(g)
                        emit_red(g, e3v.pop(g), 0)
                    if u >= 2:
                        g = u - 2
                        if g in SQ_ON_V:
                            emit_red(g, e2v[g], 1)
                        if E1_VIA_PE:
                            emit_red(g, e1v.pop(g), 1)
                        emit_red(g, e0v.pop(g), 2)

            if repeat == 1:
                body()
            else:
                hints = (mybir.EngineType.PE,) if HINT_PE else ()
                with tc.For_i(0, repeat, hint_engines=hints) as _i:
                    body()

            if PROBE["no_red"]:
                nc.vector.memset(acc_sb[:], 0.0)
            else:
                nc.vector.tensor_copy(acc_sb[:], acc_ps[:])
            nc.sync.dma_start(out=res[:], in_=res_sb[:])
            nc.sync.dma_start(out=accd[:], in_=acc_sb[:])

    nc.finalize()
    return nc


def _get_program():
    if "nc" not in _CACHE:
        _CACHE["nc"] = _build_program()
    return _CACHE["nc"]


def _core_tiles(k):
    """Per-core tile list: (rowbase, colbase, weight). Order defines t."""
    P = TS * k  # S row-block k
    Q = B + TS * (7 - k)  # T row-block 7-k
    tiles = [(P, P, 1.0), (Q, Q, 1.0)]  # SSd, TTd
    for j in range(k + 1, 8):  # SS+ (7-k tiles)
        tiles.append((P, TS * j, 2.0))
    for j in range(8 - k, 8):  # TT+ (k tiles)
        tiles.append((Q, B + TS * j, 2.0))
    for j in range(8):  # ST (8 tiles)
        tiles.append((P, B + TS * j, -2.0))
    assert len(tiles) == NTILES
    return tiles


def _host_prep(source_features, target_features):
    x = np.concatenate(
        [np.asarray(source_features, np.float32), np.asarray(target_features, np.float32)],
        axis=0,
    )  # [N, D]
    x64 = x.astype(np.float64)
    sq = np.sum(x64 * x64, axis=1)
    colsum = np.sum(x64, axis=0)
    sum_l2 = 2.0 * N * np.sum(sq) - 2.0 * np.dot(colsum, colsum)
    bandwidth = sum_l2 / (N * N - N) / (KERNEL_MUL ** (KERNEL_NUM // 2))
    a4 = 1.0 / (bandwidth * KERNEL_MUL**4)

    xt = np.ascontiguousarray(x.T).astype(np.float16)  # [D, N]
    sqf = sq.astype(np.float32)
    scale_host = np.full((128, 1), 2.0 * a4, np.float32)
    rw_host = np.zeros((128, 3), np.float16)
    for c, w in enumerate((1.0, 2.0, -2.0)):
        rw_host[:, c] = w
    AUGW = NUSLOT * 128 + NTILES * TS

    in_maps = []
    for k in range(NCORES):
        tiles = _core_tiles(k)
        rhs_host = np.empty((128, NTILES, 2, TS), np.float16)
        w_host = np.empty((128, NWB * NIB, 2, 128), np.float16)
        aug_host = np.empty((2, AUGW), np.float32)
        for t, (rb, cb, _w) in enumerate(tiles):
            rhs_host[:, t, 0, :] = xt[0:128, cb : cb + TS]
            rhs_host[:, t, 1, :] = xt[128:256, cb : cb + TS]
            v0 = NUSLOT * 128 + t * TS
            aug_host[0, v0 : v0 + TS] = -0.5 * sqf[cb : cb + TS]
            aug_host[1, v0 : v0 + TS] = 1.0
            for ib in range(NIB):
                r0 = rb + ib * 128
                us = _uslot(t, ib)
                aug_host[0, us * 128 : (us + 1) * 128] = 1.0
                aug_host[1, us * 128 : (us + 1) * 128] = -0.5 * sqf[r0 : r0 + 128]
                if 2 <= t <= 8:
                    w_host[:, (t - 2) * NIB + ib, 0, :] = xt[0:128, r0 : r0 + 128]
                    w_host[:, (t - 2) * NIB + ib, 1, :] = xt[128:256, r0 : r0 + 128]
        in_maps.append(
            {
                "xT": rhs_host,
                "wT": w_host,
                "aug2": aug_host,
                "scale": scale_host,
                "rw": rw_host,
            }
        )
    return in_maps


def _combine(results):
    total = 0.0
    for k in range(NCORES):
        r = np.asarray(results[k]["res"], np.float64).reshape(128, NTILES, NRES)
        w = np.array([w for (_rb, _cb, w) in _core_tiles(k)])
        # b4: 2 slots per tile
        total += float(np.dot(w, (r[:, :, 0] + r[:, :, 1]).sum(axis=0)))
        # b2 (and b1 in amr mode): one slot per group at its first tile
        for gi, g in enumerate(GROUPS):
            t0 = g[0]
            s = 0.0
            if gi not in SQ_ON_V:
                s += r[:, t0, 2].sum()
            if not E1_VIA_PE:
                s += r[:, t0, 3].sum()
            total += float(w[t0] * s)
        # accd: column-tiled reduce sums live in rows {0, 32, 64, 96}
        a = np.asarray(results[k]["accd"], np.float64)  # [128, TS] w-weighted
        total += float(a[0::32].sum())
    return np.float32(total / (B * B))


def kernel(source_features, target_features):
    from concourse.bass_utils import run_bass_kernel_spmd

    nc = _get_program()
    in_maps = _host_prep(source_features, target_features)
    out = run_bass_kernel_spmd(nc, in_maps, list(range(NCORES)))
    return _combine(out.results)
